# revision 1
# baseline (speedup 1.0000x reference)
"""Trainium2 Bass kernel for modulated multi-head attention (q=k=v variant).

Math (per batch b):
    style_k = s @ k_aff_w.T + k_aff_b                      # [F]
    wk[o,i] = k_weight[o,i] * style_k[i]
    demod_k[o] = rsqrt(sum_i wk[o,i]^2 + eps)
    kqv = x @ (wk * demod_k[:,None]).T                     # [N, F]
    per head h (D=64): S = q_h q_h^T / sqrt(D)  (q=k=v=kqv)
    attn = softmax(S, axis=-1); out_h = attn @ v_h
    y = modulated_linear(out, s, o_weight, o_aff_w, o_aff_b)

Sharding: data-parallel over batch B=8, one batch per NeuronCore.

Key layout decisions (all matmuls contract over the partition dim):
  * host feeds x^T, k_weight^T, o_weight^T, aff_w^T so no on-device
    transposes are ever needed.
  * kqvT   [F-part, N]  (demodulated)     -> q/k source for scores
  * kqv_v  [N-part, F]  (UN-demodulated)  -> v source for attn@v; the
    missing demod_k[i] factor is folded into the output-projection
    weight w2eff[i,o2] (i sits on partitions there).
  * scores tiles are computed row-chunk-on-partitions; since S (and
    exp(S)) are symmetric, these tiles serve directly as the moving
    operand of the attn@v matmul (outT formulation) with no transpose.
  * softmax skips the max-subtraction (scores here are ~N(0,1) with
    diagonal ~<16, far from fp32 exp overflow); exp runs on the scalar
    engine straight out of PSUM with fused per-row accumulation
    (accum_out) giving the softmax denominators for free.
  * per-head 1/rowsum lands on the free axis of the outT accumulation,
    so it is broadcast across partitions via a tiny DRAM bounce and
    folded into the PSUM->SBUF evacuation (tensor_tensor mult).
"""

import sys

if "/opt/trn_rl_repo" not in sys.path:
    sys.path.insert(0, "/opt/trn_rl_repo")

from contextlib import ExitStack

import numpy as np

import concourse.bass as bass
import concourse.bacc as bacc
import concourse.mybir as mybir
import concourse.tile as tile
from concourse.bass_utils import run_bass_kernel_spmd

P = 128          # partitions
F = 512          # hidden dim
C4 = F // P      # 4 feature chunks of 128
N = 1024         # tokens
NB = N // P      # 8 token blocks
H = 8            # heads
D = 64           # head dim
B = 8            # batch (one per core)
SCALE = 1.0 / 8.0   # 1/sqrt(D)
EPS = 1e-8

F32 = mybir.dt.float32
BF16 = mybir.dt.bfloat16   # matmul operand dtype: 1 cycle/row on the PE


def _bcast(ap_1d, parts):
    """Partition-broadcast read AP for a 1-D DRAM AP."""
    return bass.AP(
        tensor=ap_1d.tensor,
        offset=ap_1d.offset,
        ap=[[0, parts]] + [list(d) for d in ap_1d.ap],
    )


def _emit(nc, loop_reps=0):
    f32 = F32
    xT = nc.dram_tensor("xT", [F, N], BF16, kind="ExternalInput")
    s = nc.dram_tensor("s", [F], BF16, kind="ExternalInput")
    kwT = nc.dram_tensor("kwT", [F, F], BF16, kind="ExternalInput")
    kaT = nc.dram_tensor("kaT", [F, F], BF16, kind="ExternalInput")
    kb = nc.dram_tensor("kb", [F], f32, kind="ExternalInput")
    owT = nc.dram_tensor("owT", [F, F], BF16, kind="ExternalInput")
    oaT = nc.dram_tensor("oaT", [F, F], BF16, kind="ExternalInput")
    ob = nc.dram_tensor("ob", [F], f32, kind="ExternalInput")
    y = nc.dram_tensor("y", [N, F], f32, kind="ExternalOutput")

    with tile.TileContext(nc) as tc:
        if loop_reps:
            with tc.For_i(0, loop_reps, 1):
                _emit_body(nc, tc, xT, s, kwT, kaT, kb, owT, oaT, ob, y)
        else:
            _emit_body(nc, tc, xT, s, kwT, kaT, kb, owT, oaT, ob, y)


def _emit_body(nc, tc, xT, s, kwT, kaT, kb, owT, oaT, ob, y):
    f32 = F32
    Exp = mybir.ActivationFunctionType.Exp
    MULT = mybir.AluOpType.mult

    with ExitStack() as ctx:
        persist = ctx.enter_context(tc.tile_pool(name="persist", bufs=1))
        dram = ctx.enter_context(tc.tile_pool(name="dram", bufs=2, space="DRAM"))
        # One PSUM pool for the whole kernel: tag ps_s has 2 [P,N] slots
        # (4 banks), tags po0/po1 one [P,N] slot each (2+2 banks) = 8 banks.
        # Every psum use in the kernel cycles through these four slots, so no
        # phase ever stalls on a PSUM-range WAR against a closed pool.
        psum = ctx.enter_context(tc.tile_pool(name="psum", bufs=1, space="PSUM"))

        ones_col = persist.tile([P, 1], BF16)
        nc.vector.memset(ones_col, 1.0)
        # prewarm the exp table set while the input DMAs stream
        warm = persist.tile([1, 1], f32)
        nc.scalar.activation(out=warm, in_=ones_col[0:1, 0:1], func=Exp, scale=1.0)

        # k-side critical-path DMAs in dependency order (kaT feeds the style
        # matvec everything waits on). o-side weights stream later on the
        # gpsimd queue and their prep overlaps the attention phase.
        s_sb = persist.tile([P, C4], BF16)
        nc.sync.dma_start(out=s_sb, in_=s.rearrange("(c p) -> p c", p=P))
        kb_p = persist.tile([P, C4], f32)
        nc.gpsimd.dma_start(out=kb_p, in_=kb.rearrange("(c p) -> p c", p=P))

        style_k_p = persist.tile([P, C4], f32)
        style_o_p = persist.tile([P, C4], f32)
        demod_k_p = persist.tile([P, C4], f32)
        ob_p = persist.tile([P, C4], f32)
        do_b = persist.tile([P, F], f32)

        kws = persist.tile([P, C4, F], BF16)
        xT_sb = persist.tile([P, C4, N], BF16)
        w2o = persist.tile([P, C4, F], BF16)
        kqvT = persist.tile([P, C4, N], BF16)
        kqv_v = persist.tile([P, NB, F], BF16)
        aT = persist.tile([P, C4, N], BF16)
        y_acc = persist.tile([P, NB, F], f32)

        def style_partition_layout(aff_sb, bias_p, out_p):
            """out_p[p, c] = sum_j s[j] * aff_sb[j, 128c+p] + bias, via N=1
            matmuls straight into partition layout (no DRAM bounce)."""
            ps = psum.tile([P, N], f32, tag="ps_s", bufs=2)
            for co in range(C4):
                for cj in range(C4):
                    nc.tensor.matmul(
                        ps[:, co : co + 1],
                        (aff_sb[:, cj, co * P : (co + 1) * P]),
                        (s_sb[:, cj : cj + 1]),
                        start=(cj == 0), stop=(cj == C4 - 1))
            nc.vector.tensor_add(out=out_p, in0=ps[:, 0:C4], in1=bias_p)

        def demod_partition_layout(wsq, out_p, pool):
            """out_p[p, c] = rsqrt(sum_i wsq[i, 128c+p] + eps), N=1 matmuls,
            with the rsqrt entirely on the DVE (magic-constant Newton) so the
            scalar engine only ever runs exp."""
            i32 = mybir.dt.int32
            MAGIC = 0x5F3759DF
            ps = psum.tile([P, N], f32, tag="ps_s", bufs=2)
            for co in range(C4):
                for ci in range(C4):
                    nc.tensor.matmul(
                        ps[:, co : co + 1],
                        (wsq[:, ci, co * P : (co + 1) * P]),
                        (ones_col),
                        start=(ci == 0), stop=(ci == C4 - 1))
            z = pool.tile([P, C4], f32, tag="rsq_z")
            tmp = pool.tile([P, C4], f32, tag="rsq_t")
            nc.vector.tensor_scalar_add(out=z, in0=ps[:, 0:C4], scalar1=EPS)
            nc.vector.tensor_scalar(
                out=out_p.bitcast(i32), in0=z.bitcast(i32), scalar1=1,
                scalar2=None, op0=mybir.AluOpType.logical_shift_right)
            nc.vector.tensor_scalar(
                out=out_p.bitcast(i32), in0=out_p.bitcast(i32), scalar1=-1,
                scalar2=MAGIC, op0=mybir.AluOpType.mult,
                op1=mybir.AluOpType.add)
            for _ in range(3):
                nc.vector.tensor_mul(out=tmp, in0=out_p, in1=out_p)
                nc.vector.tensor_mul(out=tmp, in0=tmp, in1=z)
                nc.vector.tensor_scalar(
                    out=tmp, in0=tmp, scalar1=-0.5, scalar2=1.5,
                    op0=mybir.AluOpType.mult, op1=mybir.AluOpType.add)
                nc.vector.tensor_mul(out=out_p, in0=out_p, in1=tmp)

        # ---------- k-side prep: style_k, kws, demod_k ----------
        with tc.tile_pool(name="kprep", bufs=1) as kprep:
            kaT_sb = kprep.tile([P, C4, F], BF16, tag="kaT_sb")
            nc.sync.dma_start(out=kaT_sb, in_=kaT.rearrange("(c p) i -> p c i", p=P))
            # kws streams on the gpsimd queue, in parallel with kaT/xT on sync
            nc.gpsimd.dma_start(out=kws, in_=kwT.rearrange("(c p) o -> p c o", p=P))
            xT_r = xT.rearrange("(c p) n -> p c n", p=P)
            for nh in range(2):
                nc.sync.dma_start(out=xT_sb[:, :, nh * F : (nh + 1) * F],
                                  in_=xT_r[:, :, nh * F : (nh + 1) * F])

            style_partition_layout(kaT_sb, kb_p, style_k_p)
            for c in range(C4):
                nc.vector.tensor_scalar_mul(out=kws[:, c, :], in0=kws[:, c, :],
                                            scalar1=style_k_p[:, c : c + 1])
            sq = kprep.tile([P, C4, F], BF16, tag="sq")
            nc.vector.tensor_mul(out=sq, in0=kws, in1=kws)
            demod_partition_layout(sq, demod_k_p, kprep)

        # ---------- kqvT [o-part, n] (demodulated): q/k source ----------
        def emit_kqvT_chunk(ob_i):
            for nh in range(2):
                pt = psum.tile([P, F], f32, tag=f"po{(ob_i * 2 + nh) % 4}",
                               name="pkq")
                for c in range(C4):
                    nc.tensor.matmul(
                        pt,
                        (kws[:, c, ob_i * P : (ob_i + 1) * P]),
                        (xT_sb[:, c, nh * F : (nh + 1) * F]),
                        start=(c == 0), stop=(c == C4 - 1))
                nc.vector.tensor_scalar_mul(
                    out=kqvT[:, ob_i, nh * F : (nh + 1) * F], in0=pt,
                    scalar1=demod_k_p[:, ob_i : ob_i + 1])

        def emit_kqv_v_chunk(nb):
            pt = psum.tile([P, F], f32, tag=f"po{nb % 4}", name="pkv")
            for cc in range(C4):
                nc.tensor.matmul(
                    pt,
                    (xT_sb[:, cc, nb * P : (nb + 1) * P]),
                    (kws[:, cc, :]),
                    start=(cc == 0), stop=(cc == C4 - 1))
            nc.vector.tensor_copy(out=kqv_v[:, nb, :], in_=pt)

        # Only chunk 0 before attention: it is all pair 0 needs, so the first
        # exp fires as early as possible. Chunks 1-3 stream inside pair 0.
        emit_kqvT_chunk(0)

        def emit_oprep():
            # o-side weight prep, off the critical path: overlaps the pair-0
            # exps on PE/DVE/Pool slack.
            with tc.tile_pool(name="oprep", bufs=1) as oprep:
                nc.gpsimd.dma_start(
                    out=w2o, in_=owT.rearrange("(c p) o -> p c o", p=P))
                nc.gpsimd.dma_start(
                    out=ob_p, in_=ob.rearrange("(c p) -> p c", p=P))
                oaT_sb = oprep.tile([P, C4, F], BF16, tag="oaT_sb")
                nc.gpsimd.dma_start(
                    out=oaT_sb, in_=oaT.rearrange("(c p) i -> p c i", p=P))

                style_partition_layout(oaT_sb, ob_p, style_o_p)
                for cc in range(C4):
                    nc.vector.tensor_scalar_mul(
                        out=w2o[:, cc, :], in0=w2o[:, cc, :],
                        scalar1=style_o_p[:, cc : cc + 1])

                # demod_o, needed broadcast along free dim -> do_b [P, F]
                sqo = oprep.tile([P, C4, F], BF16, tag="sqo")
                nc.vector.tensor_mul(out=sqo, in0=w2o, in1=w2o)
                do_p = oprep.tile([P, C4], f32, tag="do_p")
                demod_partition_layout(sqo, do_p, oprep)
                d_do2 = dram.tile([F], f32, tag="d_do2")
                nc.gpsimd.dma_start(
                    out=d_do2.rearrange("(c p) -> p c", p=P), in_=do_p)
                nc.gpsimd.dma_start(out=do_b, in_=_bcast(d_do2, P))

                # w2eff = w2o * demod_k(i) * demod_o(o2)
                for cc in range(C4):
                    nc.vector.tensor_scalar_mul(
                        out=w2o[:, cc, :], in0=w2o[:, cc, :],
                        scalar1=demod_k_p[:, cc : cc + 1])
                    nc.vector.tensor_mul(out=w2o[:, cc, :],
                                         in0=w2o[:, cc, :], in1=do_b)

        # ---------- attention (head pairs) + fused output projection ----------
        def emit_ypartial(pc, nbs, with_dma):
            """Pair pc's contribution to y; emitted a pair later (or at the
            tail) so the PE never stalls on the rowsum-broadcast chain."""
            for nb in nbs:
                pt = psum.tile([P, F], f32, tag=f"po{nb % 4}", name="ypt")
                nc.tensor.matmul(
                    pt,
                    (aT[:, pc, nb * P : (nb + 1) * P]),
                    (w2o[:, pc, :]),
                    start=True, stop=True)
                if pc == 0:
                    nc.vector.tensor_copy(out=y_acc[:, nb, :], in_=pt)
                else:
                    nc.vector.tensor_add(out=y_acc[:, nb, :],
                                         in0=y_acc[:, nb, :], in1=pt)
                if with_dma:
                    eng = nc.sync if nb % 2 == 0 else nc.gpsimd
                    eng.dma_start(out=y[nb * P : (nb + 1) * P, :],
                                  in_=y_acc[:, nb, :])

        with tc.tile_pool(name="att", bufs=1) as att, \
             tc.tile_pool(name="attrs", bufs=2) as attrs:
            for pc in range(H // 2):
                c = pc
                h0, h1 = 2 * pc, 2 * pc + 1
                E0 = att.tile([P, NB, N], BF16, tag="E0")
                E1 = att.tile([P, NB, N], BF16, tag="E1")
                rows0 = attrs.tile([P, NB], f32, tag="rows0")
                rows1 = attrs.tile([P, NB], f32, tag="rows1")
                o0h = o1h = None

                def emit_attnv(mb):
                    for nh in range(2):
                        nc.tensor.matmul(
                            o0h[nh][0:D, :],
                            (kqv_v[:, mb, h0 * D : (h0 + 1) * D]),
                            (E0[:, mb, nh * F : (nh + 1) * F]),
                            start=(mb == 0), stop=(mb == NB - 1))
                        nc.tensor.matmul(
                            o1h[nh][D:P, :],
                            (kqv_v[:, mb, h1 * D : (h1 + 1) * D]),
                            (E1[:, mb, nh * F : (nh + 1) * F]),
                            start=(mb == 0), stop=(mb == NB - 1),
                            tile_position=(0, 64))

                # scores (m-chunk on partitions; symmetric) + exp with fused
                # rowsum. Head h0 lives on partitions 0:64 (PE row groups 0-1),
                # h1 on 64:128 (row groups 2-3) - the PE runs them concurrently.
                # Ready PE work (kqvT chunks, kqv_v, y-partials of the previous
                # pair, attnv of earlier mb) is interleaved into the emission so
                # the strict-FIFO PE always has work while the ACT paces exps.
                for mb in range(NB):
                    s0 = psum.tile([P, N], f32, tag="ps_s", bufs=2)
                    s1 = psum.tile([P, N], f32, tag="ps_s", bufs=2)
                    for nh in range(2):
                        nc.tensor.matmul(
                            s0[:, nh * F : (nh + 1) * F],
                            (kqvT[0:D, c, mb * P : (mb + 1) * P]),
                            (kqvT[0:D, c, nh * F : (nh + 1) * F]),
                            start=True, stop=True)
                        nc.tensor.matmul(
                            s1[:, nh * F : (nh + 1) * F],
                            (kqvT[D:P, c, mb * P : (mb + 1) * P]),
                            (kqvT[D:P, c, nh * F : (nh + 1) * F]),
                            start=True, stop=True, tile_position=(64, 0))
                    nc.scalar.activation(out=E0[:, mb, :], in_=s0, func=Exp,
                                         scale=SCALE, accum_out=rows0[:, mb : mb + 1])
                    nc.scalar.activation(out=E1[:, mb, :], in_=s1, func=Exp,
                                         scale=SCALE, accum_out=rows1[:, mb : mb + 1])

                    if pc == 0:
                        if 1 <= mb <= 3:
                            emit_kqvT_chunk(mb)
                        emit_kqv_v_chunk(mb)
                        if mb == 4:
                            emit_oprep()
                    else:
                        if mb == 0:
                            emit_ypartial(pc - 1, range(NB), with_dma=False)
                        else:
                            if mb == 1:
                                o0h = [psum.tile([P, F], f32, tag="po0", name="o00"),
                                       psum.tile([P, F], f32, tag="po1", name="o01")]
                                o1h = [psum.tile([P, F], f32, tag="po2", name="o10"),
                                       psum.tile([P, F], f32, tag="po3", name="o11")]
                            emit_attnv(mb - 1)

                if pc == 0:
                    o0h = [psum.tile([P, F], f32, tag="po0", name="o00"),
                           psum.tile([P, F], f32, tag="po1", name="o01")]
                    o1h = [psum.tile([P, F], f32, tag="po2", name="o10"),
                           psum.tile([P, F], f32, tag="po3", name="o11")]
                    for mb in range(NB):
                        emit_attnv(mb)
                else:
                    emit_attnv(NB - 1)

                # softmax denominators -> reciprocal -> [1, N] row -> broadcast
                nc.vector.reciprocal(out=rows0, in_=rows0)
                nc.vector.reciprocal(out=rows1, in_=rows1)
                d_r0 = dram.tile([N], f32, tag="d_r0")
                d_r1 = dram.tile([N], f32, tag="d_r1")
                nc.sync.dma_start(out=d_r0.rearrange("(c p) -> p c", p=P), in_=rows0)
                nc.gpsimd.dma_start(out=d_r1.rearrange("(c p) -> p c", p=P), in_=rows1)
                rs_b = attrs.tile([P, N], f32, tag="rs_b")
                nc.sync.dma_start(out=rs_b[0:D, :], in_=_bcast(d_r0, D))
                nc.gpsimd.dma_start(out=rs_b[D:P, :], in_=_bcast(d_r1, D))

                # evacuate with normalization; at the tail, pipeline the final
                # y contribution per n-half behind each evac
                for nh in range(2):
                    sl = slice(nh * F, (nh + 1) * F)
                    nc.vector.tensor_tensor(
                        aT[0:D, c, sl], o0h[nh][0:D, :], rs_b[0:D, sl], MULT)
                    nc.vector.tensor_tensor(
                        aT[D:P, c, sl], o1h[nh][D:P, :], rs_b[D:P, sl], MULT)
                    if pc == H // 2 - 1:
                        emit_ypartial(pc, range(nh * 4, nh * 4 + 4), with_dma=True)

_NC_CACHE = None


def build_nc():
    global _NC_CACHE
    if _NC_CACHE is None:
        nc = bacc.Bacc(trn_type="TRN2")
        _emit(nc)
        nc.finalize()
        _NC_CACHE = nc
    return _NC_CACHE


def make_in_maps(x, s, k_weight, k_aff_w, k_aff_b, o_weight, o_aff_w, o_aff_b):
    import ml_dtypes
    f = np.float32
    bf = ml_dtypes.bfloat16
    kwT = np.ascontiguousarray(np.asarray(k_weight, f).T.astype(bf))
    kaT = np.ascontiguousarray(np.asarray(k_aff_w, f).T.astype(bf))
    owT = np.ascontiguousarray(np.asarray(o_weight, f).T.astype(bf))
    oaT = np.ascontiguousarray(np.asarray(o_aff_w, f).T.astype(bf))
    kb = np.ascontiguousarray(np.asarray(k_aff_b, f))
    obb = np.ascontiguousarray(np.asarray(o_aff_b, f))
    return [
        {
            "xT": np.ascontiguousarray(np.asarray(x[b], f).T.astype(bf)),
            "s": np.ascontiguousarray(np.asarray(s[b], f).astype(bf)),
            "kwT": kwT, "kaT": kaT, "kb": kb,
            "owT": owT, "oaT": oaT, "ob": obb,
        }
        for b in range(B)
    ]


def kernel(x, s, k_weight, k_aff_w, k_aff_b, o_weight, o_aff_w, o_aff_b):
    assert x.shape == (B, N, F), x.shape
    nc = build_nc()
    in_maps = make_in_maps(x, s, k_weight, k_aff_w, k_aff_b,
                           o_weight, o_aff_w, o_aff_b)
    res = run_bass_kernel_spmd(nc, in_maps, list(range(B)))
    return np.stack([res.results[b]["y"] for b in range(B)], axis=0)



# revision 14
# speedup vs baseline: 2.0678x; 2.0678x over previous
"""Trainium2 Bass kernel for modulated multi-head attention (q=k=v variant).

Math (per batch b):
    style_k = s @ k_aff_w.T + k_aff_b                      # [F]
    wk[o,i] = k_weight[o,i] * style_k[i]
    demod_k[o] = rsqrt(sum_i wk[o,i]^2 + eps)
    kqv = x @ (wk * demod_k[:,None]).T                     # [N, F]
    per head h (D=64): S = q_h q_h^T / sqrt(D)  (q=k=v=kqv)
    attn = softmax(S, axis=-1); out_h = attn @ v_h
    y = modulated_linear(out, s, o_weight, o_aff_w, o_aff_b)

Sharding: data-parallel over batch B=8, one batch per NeuronCore.

Key layout decisions (all matmuls contract over the partition dim):
  * host feeds x^T, k_weight^T, o_weight^T, aff_w^T so no on-device
    transposes are ever needed.
  * kqvT   [F-part, N]  (demodulated)     -> q/k source for scores
  * kqv_v  [N-part, F]  (UN-demodulated)  -> v source for attn@v; the
    missing demod_k[i] factor is folded into the output-projection
    weight w2eff[i,o2] (i sits on partitions there).
  * scores tiles are computed row-chunk-on-partitions; since S (and
    exp(S)) are symmetric, these tiles serve directly as the moving
    operand of the attn@v matmul (outT formulation) with no transpose.
  * softmax skips the max-subtraction (scores here are ~N(0,1) with
    diagonal ~<16, far from fp32 exp overflow); exp runs on the scalar
    engine straight out of PSUM with fused per-row accumulation
    (accum_out) giving the softmax denominators for free.
  * per-head 1/rowsum lands on the free axis of the outT accumulation,
    so it is broadcast across partitions via a tiny DRAM bounce and
    folded into the PSUM->SBUF evacuation (tensor_tensor mult).
"""

import sys

if "/opt/trn_rl_repo" not in sys.path:
    sys.path.insert(0, "/opt/trn_rl_repo")

from contextlib import ExitStack

import numpy as np

import concourse.bass as bass
import concourse.bacc as bacc
import concourse.mybir as mybir
import concourse.tile as tile
from concourse.bass_utils import run_bass_kernel_spmd

P = 128          # partitions
F = 512          # hidden dim
C4 = F // P      # 4 feature chunks of 128
N = 1024         # tokens
NB = N // P      # 8 token blocks
H = 8            # heads
D = 64           # head dim
B = 8            # batch (one per core)
SCALE = 1.0 / 8.0   # 1/sqrt(D)
EPS = 1e-8

F32 = mybir.dt.float32
BF16 = mybir.dt.bfloat16   # matmul operand dtype: 1 cycle/row on the PE


def _bcast(ap_1d, parts):
    """Partition-broadcast read AP for a 1-D DRAM AP."""
    return bass.AP(
        tensor=ap_1d.tensor,
        offset=ap_1d.offset,
        ap=[[0, parts]] + [list(d) for d in ap_1d.ap],
    )


def _emit(nc, loop_reps=0):
    f32 = F32
    xT = nc.dram_tensor("xT", [F, N], BF16, kind="ExternalInput")
    s = nc.dram_tensor("s", [F], BF16, kind="ExternalInput")
    kwT = nc.dram_tensor("kwT", [F, F], BF16, kind="ExternalInput")
    kaT = nc.dram_tensor("kaT", [F, F], BF16, kind="ExternalInput")
    kb = nc.dram_tensor("kb", [F], f32, kind="ExternalInput")
    owT = nc.dram_tensor("owT", [F, F], BF16, kind="ExternalInput")
    oaT = nc.dram_tensor("oaT", [F, F], BF16, kind="ExternalInput")
    ob = nc.dram_tensor("ob", [F], f32, kind="ExternalInput")
    y = nc.dram_tensor("y", [N, F], f32, kind="ExternalOutput")

    with tile.TileContext(nc) as tc:
        if loop_reps:
            with tc.For_i(0, loop_reps, 1):
                _emit_body(nc, tc, xT, s, kwT, kaT, kb, owT, oaT, ob, y)
        else:
            _emit_body(nc, tc, xT, s, kwT, kaT, kb, owT, oaT, ob, y)


def _emit_body(nc, tc, xT, s, kwT, kaT, kb, owT, oaT, ob, y):
    f32 = F32
    Exp = mybir.ActivationFunctionType.Exp
    MULT = mybir.AluOpType.mult

    with ExitStack() as ctx:
        persist = ctx.enter_context(tc.tile_pool(name="persist", bufs=1))
        dram = ctx.enter_context(tc.tile_pool(name="dram", bufs=2, space="DRAM"))
        # One PSUM pool for the whole kernel: tag ps_s has 2 [P,N] slots
        # (4 banks), tags po0/po1 one [P,N] slot each (2+2 banks) = 8 banks.
        # Every psum use in the kernel cycles through these four slots, so no
        # phase ever stalls on a PSUM-range WAR against a closed pool.
        psum = ctx.enter_context(tc.tile_pool(name="psum", bufs=1, space="PSUM"))

        ones_col = persist.tile([P, 1], BF16)
        nc.vector.memset(ones_col, 1.0)
        # prewarm the exp table set while the input DMAs stream
        warm = persist.tile([1, 1], f32)
        nc.scalar.activation(out=warm, in_=ones_col[0:1, 0:1], func=Exp, scale=1.0)

        # k-side critical-path DMAs in dependency order (kaT feeds the style
        # matvec everything waits on). o-side weights stream later on the
        # gpsimd queue and their prep overlaps the attention phase.
        s_sb = persist.tile([P, C4], BF16)
        nc.sync.dma_start(out=s_sb, in_=s.rearrange("(c p) -> p c", p=P))
        kb_p = persist.tile([P, C4], f32)
        nc.gpsimd.dma_start(out=kb_p, in_=kb.rearrange("(c p) -> p c", p=P))

        style_k_p = persist.tile([P, C4], f32)
        style_o_p = persist.tile([P, C4], f32)
        demod_k_p = persist.tile([P, C4], f32)
        ob_p = persist.tile([P, C4], f32)
        do_b = persist.tile([P, F], f32)

        kws = persist.tile([P, C4, F], BF16)
        xT_sb = persist.tile([P, C4, N], BF16)
        w2o = persist.tile([P, C4, F], BF16)
        kqvT = persist.tile([P, C4, N], BF16)
        kqv_v = persist.tile([P, NB, F], BF16)
        aT = persist.tile([P, C4, N], BF16)
        y_acc = persist.tile([P, NB, F], f32)

        def style_partition_layout(aff_sb, bias_p, out_p, ptag):
            """out_p[p, c] = sum_j s[j] * aff_sb[j, 128c+p] + bias, via N=1
            matmuls straight into partition layout (no DRAM bounce)."""
            ps = psum.tile([P, N], f32, tag="ps_s", bufs=2)
            for co in range(C4):
                for cj in range(C4):
                    nc.tensor.matmul(
                        ps[:, co : co + 1],
                        (aff_sb[:, cj, co * P : (co + 1) * P]),
                        (s_sb[:, cj : cj + 1]),
                        start=(cj == 0), stop=(cj == C4 - 1))
            nc.vector.tensor_add(out=out_p, in0=ps[:, 0:C4], in1=bias_p)

        def demod_partition_layout(wsq, out_p, pool, ptag):
            """out_p[p, c] = rsqrt(sum_i wsq[i, 128c+p] + eps), N=1 matmuls,
            with the rsqrt entirely on the DVE (magic-constant Newton) so the
            scalar engine only ever runs exp."""
            i32 = mybir.dt.int32
            MAGIC = 0x5F3759DF
            ps = psum.tile([P, N], f32, tag="ps_s", bufs=2)
            for co in range(C4):
                for ci in range(C4):
                    nc.tensor.matmul(
                        ps[:, co : co + 1],
                        (wsq[:, ci, co * P : (co + 1) * P]),
                        (ones_col),
                        start=(ci == 0), stop=(ci == C4 - 1))
            z = pool.tile([P, C4], f32, tag="rsq_z")
            tmp = pool.tile([P, C4], f32, tag="rsq_t")
            nc.vector.tensor_scalar_add(out=z, in0=ps[:, 0:C4], scalar1=EPS)
            nc.vector.tensor_scalar(
                out=out_p.bitcast(i32), in0=z.bitcast(i32), scalar1=1,
                scalar2=None, op0=mybir.AluOpType.logical_shift_right)
            nc.vector.tensor_scalar(
                out=out_p.bitcast(i32), in0=out_p.bitcast(i32), scalar1=-1,
                scalar2=MAGIC, op0=mybir.AluOpType.mult,
                op1=mybir.AluOpType.add)
            for _ in range(3):
                nc.vector.tensor_mul(out=tmp, in0=out_p, in1=out_p)
                nc.vector.tensor_mul(out=tmp, in0=tmp, in1=z)
                nc.vector.tensor_scalar(
                    out=tmp, in0=tmp, scalar1=-0.5, scalar2=1.5,
                    op0=mybir.AluOpType.mult, op1=mybir.AluOpType.add)
                nc.vector.tensor_mul(out=out_p, in0=out_p, in1=tmp)

        # ---------- k-side prep: style_k, kws, demod_k ----------
        with tc.tile_pool(name="kprep", bufs=1) as kprep:
            kaT_sb = kprep.tile([P, C4, F], BF16, tag="kaT_sb")
            nc.sync.dma_start(out=kaT_sb, in_=kaT.rearrange("(c p) i -> p c i", p=P))
            # kws streams on the gpsimd queue, in parallel with kaT/xT on sync
            nc.gpsimd.dma_start(out=kws, in_=kwT.rearrange("(c p) o -> p c o", p=P))
            xT_r = xT.rearrange("(c p) n -> p c n", p=P)
            for nh in range(2):
                nc.sync.dma_start(out=xT_sb[:, :, nh * F : (nh + 1) * F],
                                  in_=xT_r[:, :, nh * F : (nh + 1) * F])

            style_partition_layout(kaT_sb, kb_p, style_k_p, None)
            for c in range(C4):
                nc.vector.tensor_scalar_mul(out=kws[:, c, :], in0=kws[:, c, :],
                                            scalar1=style_k_p[:, c : c + 1])
            sq = kprep.tile([P, C4, F], BF16, tag="sq")
            nc.vector.tensor_mul(out=sq, in0=kws, in1=kws)
            demod_partition_layout(sq, demod_k_p, kprep, None)

        # ---------- kqvT [o-part, n] (demodulated): q/k source ----------
        def emit_kqvT_chunk(ob_i):
            for nh in range(2):
                pt = psum.tile([P, F], f32, tag=f"po{(ob_i * 2 + nh) % 4}",
                               name="pkq")
                for c in range(C4):
                    nc.tensor.matmul(
                        pt,
                        (kws[:, c, ob_i * P : (ob_i + 1) * P]),
                        (xT_sb[:, c, nh * F : (nh + 1) * F]),
                        start=(c == 0), stop=(c == C4 - 1))
                nc.vector.tensor_scalar_mul(
                    out=kqvT[:, ob_i, nh * F : (nh + 1) * F], in0=pt,
                    scalar1=demod_k_p[:, ob_i : ob_i + 1])

        def emit_kqv_v_chunk(nb):
            pt = psum.tile([P, F], f32, tag=f"po{nb % 4}", name="pkv")
            for cc in range(C4):
                nc.tensor.matmul(
                    pt,
                    (xT_sb[:, cc, nb * P : (nb + 1) * P]),
                    (kws[:, cc, :]),
                    start=(cc == 0), stop=(cc == C4 - 1))
            nc.vector.tensor_copy(out=kqv_v[:, nb, :], in_=pt)

        # Only chunk 0 before attention: it is all pair 0 needs, so the first
        # exp fires as early as possible. Chunks 1-3 stream inside pair 0.
        emit_kqvT_chunk(0)

        def emit_oprep():
            # o-side weight prep, off the critical path: overlaps the pair-0
            # exps on PE/DVE/Pool slack. Big streams stay on the gpsimd
            # queue; the latency-critical rowsum bounces own sync/scalar.
            with tc.tile_pool(name="oprep", bufs=1) as oprep:
                nc.gpsimd.dma_start(
                    out=w2o, in_=owT.rearrange("(c p) o -> p c o", p=P))
                nc.gpsimd.dma_start(
                    out=ob_p, in_=ob.rearrange("(c p) -> p c", p=P))
                oaT_sb = oprep.tile([P, C4, F], BF16, tag="oaT_sb")
                nc.gpsimd.dma_start(
                    out=oaT_sb, in_=oaT.rearrange("(c p) i -> p c i", p=P))

                style_partition_layout(oaT_sb, ob_p, style_o_p, None)
                for cc in range(C4):
                    nc.vector.tensor_scalar_mul(
                        out=w2o[:, cc, :], in0=w2o[:, cc, :],
                        scalar1=style_o_p[:, cc : cc + 1])

                # demod_o, needed broadcast along free dim -> do_b [P, F]
                sqo = oprep.tile([P, C4, F], BF16, tag="sqo")
                nc.vector.tensor_mul(out=sqo, in0=w2o, in1=w2o)
                do_p = oprep.tile([P, C4], f32, tag="do_p")
                demod_partition_layout(sqo, do_p, oprep, None)
                d_do2 = dram.tile([F], f32, tag="d_do2")
                nc.gpsimd.dma_start(
                    out=d_do2.rearrange("(c p) -> p c", p=P), in_=do_p)
                nc.gpsimd.dma_start(out=do_b, in_=_bcast(d_do2, P))

                # w2eff = w2o * demod_k(i) * demod_o(o2)
                for cc in range(C4):
                    nc.vector.tensor_scalar_mul(
                        out=w2o[:, cc, :], in0=w2o[:, cc, :],
                        scalar1=demod_k_p[:, cc : cc + 1])
                    nc.vector.tensor_mul(out=w2o[:, cc, :],
                                         in0=w2o[:, cc, :], in1=do_b)

        # ---------- attention (head pairs) + fused output projection ----------
        def emit_ypartial(pc, nbs, with_dma):
            """Pair pc's contribution to y; emitted a pair later (or at the
            tail) so the PE never stalls on the rowsum-broadcast chain."""
            for nb in nbs:
                pt = psum.tile([P, F], f32, tag=f"po{nb % 4}", name="ypt")
                nc.tensor.matmul(
                    pt,
                    (aT[:, pc, nb * P : (nb + 1) * P]),
                    (w2o[:, pc, :]),
                    start=True, stop=True)
                if pc == 0:
                    nc.vector.tensor_copy(out=y_acc[:, nb, :], in_=pt)
                else:
                    nc.vector.tensor_add(out=y_acc[:, nb, :],
                                         in0=y_acc[:, nb, :], in1=pt)
                if with_dma:
                    eng = nc.sync if nb % 2 == 0 else nc.gpsimd
                    eng.dma_start(out=y[nb * P : (nb + 1) * P, :],
                                  in_=y_acc[:, nb, :])

        with tc.tile_pool(name="att", bufs=1) as att, \
             tc.tile_pool(name="attrs", bufs=2) as attrs:
            for pc in range(H // 2):
                c = pc
                h0, h1 = 2 * pc, 2 * pc + 1
                E0 = att.tile([P, NB, N], BF16, tag="E0")
                E1 = att.tile([P, NB, N], BF16, tag="E1")
                rows0 = attrs.tile([P, NB], f32, tag="rows0")
                rows1 = attrs.tile([P, NB], f32, tag="rows1")
                o0h = o1h = None

                def emit_attnv(mb):
                    for nh in range(2):
                        nc.tensor.matmul(
                            o0h[nh][0:D, :],
                            (kqv_v[:, mb, h0 * D : (h0 + 1) * D]),
                            (E0[:, mb, nh * F : (nh + 1) * F]),
                            start=(mb == 0), stop=(mb == NB - 1))
                        nc.tensor.matmul(
                            o1h[nh][D:P, :],
                            (kqv_v[:, mb, h1 * D : (h1 + 1) * D]),
                            (E1[:, mb, nh * F : (nh + 1) * F]),
                            start=(mb == 0), stop=(mb == NB - 1),
                            tile_position=(0, 64))

                # scores (m-chunk on partitions; symmetric) + exp with fused
                # rowsum. Head h0 lives on partitions 0:64 (PE row groups 0-1),
                # h1 on 64:128 (row groups 2-3) - the PE runs them concurrently.
                # Ready PE work (kqvT chunks, kqv_v, y-partials of the previous
                # pair, attnv of earlier mb) is interleaved into the emission so
                # the strict-FIFO PE always has work while the ACT paces exps.
                for mb in range(NB):
                    s0 = psum.tile([P, N], f32, tag="ps_s", bufs=2)
                    s1 = psum.tile([P, N], f32, tag="ps_s", bufs=2)
                    for nh in range(2):
                        nc.tensor.matmul(
                            s0[:, nh * F : (nh + 1) * F],
                            (kqvT[0:D, c, mb * P : (mb + 1) * P]),
                            (kqvT[0:D, c, nh * F : (nh + 1) * F]),
                            start=True, stop=True)
                        nc.tensor.matmul(
                            s1[:, nh * F : (nh + 1) * F],
                            (kqvT[D:P, c, mb * P : (mb + 1) * P]),
                            (kqvT[D:P, c, nh * F : (nh + 1) * F]),
                            start=True, stop=True, tile_position=(64, 0))
                    nc.scalar.activation(out=E0[:, mb, :], in_=s0, func=Exp,
                                         scale=SCALE, accum_out=rows0[:, mb : mb + 1])
                    nc.scalar.activation(out=E1[:, mb, :], in_=s1, func=Exp,
                                         scale=SCALE, accum_out=rows1[:, mb : mb + 1])

                    if pc == 0:
                        if 1 <= mb <= 3:
                            emit_kqvT_chunk(mb)
                        emit_kqv_v_chunk(mb)
                        if mb == 4:
                            emit_oprep()
                    else:
                        if mb == 0:
                            emit_ypartial(pc - 1, range(NB), with_dma=False)
                        else:
                            if mb == 1:
                                o0h = [psum.tile([P, F], f32, tag="po0", name="o00"),
                                       psum.tile([P, F], f32, tag="po1", name="o01")]
                                o1h = [psum.tile([P, F], f32, tag="po2", name="o10"),
                                       psum.tile([P, F], f32, tag="po3", name="o11")]
                            emit_attnv(mb - 1)

                if pc == 0:
                    o0h = [psum.tile([P, F], f32, tag="po0", name="o00"),
                           psum.tile([P, F], f32, tag="po1", name="o01")]
                    o1h = [psum.tile([P, F], f32, tag="po2", name="o10"),
                           psum.tile([P, F], f32, tag="po3", name="o11")]
                    for mb in range(NB):
                        emit_attnv(mb)
                else:
                    emit_attnv(NB - 1)

                # softmax denominators -> reciprocal -> [1, N] row -> broadcast
                nc.vector.reciprocal(out=rows0, in_=rows0)
                nc.vector.reciprocal(out=rows1, in_=rows1)
                d_r0 = dram.tile([N], f32, tag="d_r0")
                d_r1 = dram.tile([N], f32, tag="d_r1")
                nc.sync.dma_start(out=d_r0.rearrange("(c p) -> p c", p=P), in_=rows0)
                nc.gpsimd.dma_start(out=d_r1.rearrange("(c p) -> p c", p=P), in_=rows1)
                rs_b = attrs.tile([P, N], f32, tag="rs_b")
                nc.sync.dma_start(out=rs_b[0:D, :], in_=_bcast(d_r0, D))
                nc.gpsimd.dma_start(out=rs_b[D:P, :], in_=_bcast(d_r1, D))

                # evacuate with normalization; at the tail, pipeline the final
                # y contribution per n-half behind each evac
                for nh in range(2):
                    sl = slice(nh * F, (nh + 1) * F)
                    nc.vector.tensor_tensor(
                        aT[0:D, c, sl], o0h[nh][0:D, :], rs_b[0:D, sl], MULT)
                    nc.vector.tensor_tensor(
                        aT[D:P, c, sl], o1h[nh][D:P, :], rs_b[D:P, sl], MULT)
                    if pc == H // 2 - 1:
                        emit_ypartial(pc, range(nh * 4, nh * 4 + 4), with_dma=True)

_NC_CACHE = None


def build_nc():
    global _NC_CACHE
    if _NC_CACHE is None:
        nc = bacc.Bacc(trn_type="TRN2")
        _emit(nc)
        nc.finalize()
        _NC_CACHE = nc
    return _NC_CACHE


def make_in_maps(x, s, k_weight, k_aff_w, k_aff_b, o_weight, o_aff_w, o_aff_b):
    import ml_dtypes
    f = np.float32
    bf = ml_dtypes.bfloat16
    kwT = np.ascontiguousarray(np.asarray(k_weight, f).T.astype(bf))
    kaT = np.ascontiguousarray(np.asarray(k_aff_w, f).T.astype(bf))
    owT = np.ascontiguousarray(np.asarray(o_weight, f).T.astype(bf))
    oaT = np.ascontiguousarray(np.asarray(o_aff_w, f).T.astype(bf))
    kb = np.ascontiguousarray(np.asarray(k_aff_b, f))
    obb = np.ascontiguousarray(np.asarray(o_aff_b, f))
    return [
        {
            "xT": np.ascontiguousarray(np.asarray(x[b], f).T.astype(bf)),
            "s": np.ascontiguousarray(np.asarray(s[b], f).astype(bf)),
            "kwT": kwT, "kaT": kaT, "kb": kb,
            "owT": owT, "oaT": oaT, "ob": obb,
        }
        for b in range(B)
    ]


def kernel(x, s, k_weight, k_aff_w, k_aff_b, o_weight, o_aff_w, o_aff_b):
    assert x.shape == (B, N, F), x.shape
    nc = build_nc()
    in_maps = make_in_maps(x, s, k_weight, k_aff_w, k_aff_b,
                           o_weight, o_aff_w, o_aff_b)
    res = run_bass_kernel_spmd(nc, in_maps, list(range(B)))
    return np.stack([res.results[b]["y"] for b in range(B)], axis=0)

